# revision 35
# baseline (speedup 1.0000x reference)
"""BERT self-attention (flash-style) Trainium2 Bass kernel.

Full inputs -> full output. Shards data-parallel over batch: batch element i
runs on NeuronCore i (B == 8 == n_cores), no collectives.

Host-side prep (cheap numpy): pack every constant into the exact per-partition
layout the SBUF tiles use, so each DMA is one long contiguous run per
partition (128 descriptors instead of thousands -- descriptor issue rate, not
HBM bandwidth, dominated the old startup). Fold the 1/sqrt(d) scale into the
q block of Wqkv, turn the key-padding mask into an additive exp bias
(0 / -30000) and the query mask into a 0/1 multiplier.

On-chip per core (S=1024, E=768, H=12, D=64):
  Scores tiles pack both heads of a pair: [sk=128, head 2p sq-half | head
  2p+1 sq-half]; the two 64-contraction matmuls run concurrently on PE
  row-tiles 0/64 (tile_position), and a single Exp activation (N=1024)
  covers both. Scores tiles are emitted in 2-tile bursts (both psum ring
  slots) to amortize the PE mode-switch cost between row-tiled score
  matmuls and full-array matmuls.
  ctx accumulated per (head, sq-half) in [128, 512] psum with a 128-wide
  lhsT head slot [ones, zeros(63), v(64)]: psum row 0 = softmax denominator
  (free via the ones column), rows 64:128 = ctx. The denominator reciprocal
  (reciprocal_approx_fast, fp32, straight from psum row 0) is broadcast over
  all partitions by gpsimd partition_broadcast (attn library) -- the PE
  rank-1 broadcast matmul and its psum-ring traffic are gone, and the
  fp32 recip path cuts rel err to 7.0e-3. VectorE multiplies rows 64:128
  and the head-even half is DMA-shifted down to its ctxT partition range.
"""

import sys

if "/opt/trn_rl_repo" not in sys.path:
    sys.path.insert(0, "/opt/trn_rl_repo")

import numpy as np
import ml_dtypes

import concourse.bass as bass
import concourse.bacc as bacc
import concourse.tile as tile
from concourse import mybir
from concourse.bass_utils import run_bass_kernel_spmd
from concourse import library_config

B, S, E, H = 8, 1024, 768, 12
D = E // H            # 64
NP = 128              # SBUF/PSUM partitions
EC = E // NP          # 6 e-chunks (contraction chunks)
SC = S // NP          # 8 sequence chunks
NPAIR = H // 2        # 6 head pairs
HALF = S // 2         # 512
BF16 = mybir.dt.bfloat16
F32 = mybir.dt.float32
F16 = mybir.dt.float16
EXP = mybir.ActivationFunctionType.Exp
MASK_NEG = -30000.0


def _drive(*gens):
    """Round-robin generators to completion (one yield-slice each per round)."""
    gens = [g for g in gens if g is not None]
    while gens:
        nxt = []
        for g in gens:
            try:
                next(g)
                nxt.append(g)
            except StopIteration:
                continue
        gens = nxt


def _drive_rot(pace, fillers, per_round=1):
    """Advance `pace` every round but only `per_round` filler slices per
    round (rotating) -- keeps the tensor queue balanced against ScalarE."""
    fillers = [g for g in fillers if g is not None]
    i = 0
    while pace is not None or fillers:
        if pace is not None:
            try:
                next(pace)
            except StopIteration:
                pace = None
        for _ in range(per_round):
            if not fillers:
                break
            g = fillers[i % len(fillers)]
            try:
                next(g)
                i += 1
            except StopIteration:
                fillers.remove(g)


def _seq(*gens):
    """Chain generators sequentially (for work sharing one psum ring)."""
    for g in gens:
        yield from g


def _body(tc, xp, wqk0, wqkr, wvp, wop, bqk, bo, kq, out, with_bias):
    nc = tc.nc

    with tc.tile_pool(name="const", bufs=1) as const:
        # ---- persistent SBUF state. Every DMA below is one contiguous run
        # per partition (host pre-packed). Order = consumption order: the
        # first q/k pair columns + x chunks first so qkT(0) starts ~3us in.
        # critical-path DMAs split across engine queues: scalar/vector reach
        # their instruction streams ~1us before sync, and parallel issue
        # shortens the descriptor-generation serial path.
        wqk0_t = const.tile([NP, EC, 2, NP], BF16, name="wqk0")
        for i, lo in enumerate((0, 32, 64, 96)):
            eng = nc.scalar if i % 2 else nc.sync
            eng.dma_start(out=wqk0_t[lo:lo + 32], in_=wqk0[lo:lo + 32, :])
        kq_sb = const.tile([NP, 2 * SC], F32)      # [kb bias | qm] packed
        nc.scalar.dma_start(out=kq_sb, in_=kq[:, :])
        xtb = const.tile([NP, EC, S], BF16, name="xtb")
        for k in range(EC):
            for i, lo in enumerate((0, 64)):
                eng = nc.scalar if (2 * k + i) % 2 else nc.sync
                eng.dma_start(out=xtb[lo:lo + 64, k, :],
                              in_=xp[lo:lo + 64, k * S:(k + 1) * S])
        kb_sb = kq_sb[:, 0:SC]
        qm_sb = kq_sb[:, SC:2 * SC]
        # remaining q/k pair columns (pairs 1-5), v groups, out_w -- streamed
        # during compute in consumption order.
        wqkr_t = const.tile([NP, 5, EC, 2, NP], BF16, name="wqkr")
        for pj in range(5):
            nc.sync.dma_start(
                out=wqkr_t[:, pj], in_=wqkr[:, pj * 1536:(pj + 1) * 1536])
        wvt = const.tile([NP, 3, EC, 256], BF16, name="wvt")
        for g in range(3):
            nc.sync.dma_start(
                out=wvt[:, g], in_=wvp[:, g * 1536:(g + 1) * 1536])
        wotb = const.tile([NP, EC, E], BF16, name="wotb")
        for h in range(2):
            nc.sync.dma_start(
                out=wotb[:, 3 * h:3 * h + 3, :],
                in_=wop[:, h * 2304:(h + 1) * 2304])
        ones_bf = const.tile([NP, 64], BF16)       # lhsT for rank-1 broadcast mm
        nc.vector.memset(ones_bf, 1.0)
        wu = const.tile([NP, 256], BF16)           # PE warm-up operand
        nc.vector.memset(wu, 1.0)
        scr = const.tile([NP, 2], F32)             # activation-table warm scratch
        # warm the Exp activation table during the DMA wait so the first
        # real EXP doesn't pay the ~1.3us table load.
        nc.scalar.activation(scr[0:1, 0:1], wu[0:1, 0:1], EXP)
        # gpsimd library with partition_broadcast (softmax denom broadcast)
        nc.gpsimd.load_library(library_config.attn)

        xt_k = [xtb[:, k, :] for k in range(EC)]
        wo_k = [wotb[:, k, :] for k in range(EC)]

        def q_lhsT(j, k):
            # lhsT columns for q pair j (j<NPAIR) or k pair j-NPAIR
            qk = 1 if j >= NPAIR else 0
            pj = j - NPAIR if j >= NPAIR else j
            if pj == 0:
                return wqk0_t[:, k, qk, :]
            return wqkr_t[:, pj - 1, k, qk, :]

        def v_rhs(g, k):
            return wvt[:, g, k, :]
        if with_bias:
            bq_sb = const.tile([NP, 3 * E // NP], F32)
            nc.sync.dma_start(out=bq_sb, in_=bqk.rearrange("(c p) -> p c", p=NP))
            bvcol = const.tile([NP, H], F32)       # v bias, col h = bias[2E+64h+p]
            nc.sync.dma_start(
                out=bvcol[64:128, :],
                in_=bass.AP(tensor=bqk, offset=2 * E, ap=[[1, 64], [64, H]]),
            )
            bo_bc = const.tile([NP, E], F32)       # out bias broadcast
            nc.sync.dma_start(
                out=bo_bc, in_=bass.AP(tensor=bo, offset=0, ap=[[0, NP], [1, E]])
            )
        else:
            bq_sb = bvcol = bo_bc = None

        _compute(tc, nc, with_bias, xt_k, q_lhsT, v_rhs, wo_k, kb_sb, qm_sb,
                 ones_bf, wu, scr, out, bq_sb, bvcol, bo_bc)


def _compute(tc, nc, with_bias, xt_k, q_lhsT, v_rhs, wo_k, kb_sb, qm_sb,
             ones_bf, wu, scr, out, bq_sb, bvcol, bo_bc):
    with tc.tile_pool(name="work", bufs=1) as work:
        # qT/kT: [128, j, s] bf16; partition = f within chunk. j=0..5 q pairs
        # (heads 2j,2j+1 at partitions 0-63 / 64-127), j=6..11 k pairs.
        qkT = work.tile([NP, H, S], BF16)
        # v (+ per-head denominator column): s-chunk m on partitions.
        # Head slot of 128 columns: [ones, zeros(63), v(64)]. The ones
        # column FIRST makes the ctx matmul emit the softmax denominator as
        # psum ROW 0, where gpsimd partition_broadcast can read it (the
        # ucode broadcasts partition 0 -- AP base partitions are ignored);
        # the zero pad parks the ctx values at rows 64:128, a legal
        # 64-aligned partition base for the DVE multiply. Matmul cost is
        # unchanged (time scales with N, not with lhsT width).
        vsb = work.tile([NP, SC, H * NP], BF16)
        vsb_4d = vsb.rearrange("p m (h t) -> p m h t", t=NP)
        nc.vector.memset(vsb_4d[:, :, :, 0:1], 1.0)
        nc.vector.memset(vsb_4d[:, :, :, 1:64], 0.0)
        # ctx.T: pair j -> partitions 0:64 head 2j, 64:128 head 2j+1; e-chunk j.
        ctxT = work.tile([NP, EC, S], BF16)

        with tc.tile_pool(name="norm", bufs=3) as normp, \
             tc.tile_pool(name="exps", bufs=31) as exps, \
             tc.tile_pool(name="osb", bufs=2) as outp, \
             tc.tile_pool(name="ps_sc", bufs=2, space="PSUM") as ps_sc, \
             tc.tile_pool(name="ps_qk", bufs=1, space="PSUM") as ps_qk, \
             tc.tile_pool(name="ps_ctx", bufs=2, space="PSUM") as ps_ctx:

            # psum budget: sc 2x[128,1024]=4 banks, qk 1x[128,1024]=2,
            # ctx 2x[128,512]=2 (ring shared by ctx accum + bcast tiles).

            def gen_warmup(n=40):
                # keep the PE busy through the initial DMA wait so HAM is at
                # K=8/8 when the first real matmul issues.
                pw = ps_sc.tile([NP, S], F32, tag="sc")
                for _ in range(n):
                    nc.tensor.matmul(pw[:, 0:256], lhsT=wu[:, 0:NP], rhs=wu,
                                     start=True, stop=True)
                    yield

            def gen_v2(g):
                # v columns for heads 4g..4g+3 (pairs 2g, 2g+1); 4 m-chunks
                # per [128, 1024] psum tile (m-chunk -> 256-col slice).
                for m0 in (0, 4):
                    pv = ps_qk.tile([NP, S], F32, tag="qk")
                    for m in range(m0, m0 + 4):
                        for k in range(EC):
                            nc.tensor.matmul(
                                pv[:, (m - m0) * 256:(m - m0 + 1) * 256],
                                lhsT=xt_k[k][:, m * NP:(m + 1) * NP],
                                rhs=v_rhs(g, k),
                                start=(k == 0), stop=(k == EC - 1),
                            )
                        yield
                    pv_v = pv.rearrange("p (m h d) -> p m h d", h=4, d=D)
                    nc.vector.tensor_copy(
                        out=vsb_4d[:, m0:m0 + 4, 4 * g:4 * g + 4, 64:128],
                        in_=pv_v)
                    yield

            def gen_qkT(j, ring):
                pq = ring.tile([NP, S], F32, tag="sc" if ring is ps_sc else "qk")
                for k in range(EC):
                    st, sp = (k == 0), (k == EC - 1)
                    for n in (0, 512):
                        nc.tensor.matmul(
                            pq[:, n:n + 512],
                            lhsT=q_lhsT(j, k),
                            rhs=xt_k[k][:, n:n + 512],
                            start=st, stop=sp,
                        )
                    if k % 2 == 1:
                        yield
                nc.vector.tensor_copy(out=qkT[:, j, :], in_=pq)
                if with_bias:
                    nc.vector.tensor_scalar_add(
                        out=qkT[:, j, :], in0=qkT[:, j, :],
                        scalar1=bq_sb[:, j:j + 1],
                    )
                yield

            pair_exps = {}

            def _emit_scores_tile(p, ch, tiles):
                c, h = ch
                st = ps_sc.tile([NP, S], F32, tag="sc")
                nc.tensor.matmul(
                    st[:, 0:512],
                    lhsT=qkT[0:64, NPAIR + p, c * NP:(c + 1) * NP],
                    rhs=qkT[0:64, p, h * HALF:(h + 1) * HALF],
                    start=True, stop=True, tile_position=(0, 0),
                )
                nc.tensor.matmul(
                    st[:, 512:1024],
                    lhsT=qkT[64:128, NPAIR + p, c * NP:(c + 1) * NP],
                    rhs=qkT[64:128, p, h * HALF:(h + 1) * HALF],
                    start=True, stop=True, tile_position=(64, 0),
                )
                e = exps.tile([NP, S], BF16, tag="exp")
                nc.scalar.activation(e, st, EXP, bias=kb_sb[:, c:c + 1])
                tiles[(c, h)] = e

            def gen_scores(p, t0=0, t1=2 * SC, order=None):
                # tile (c, h): [sk chunk c, head 2p sq-half h | head 2p+1];
                # the two matmuls run concurrently on PE row-tiles 0/64.
                # Emit tiles in 2-tile bursts (both psum ring slots) so the
                # row-tiled mode switch is paid once per burst, not per tile.
                # `order` overrides the default c-major (c, h) sequence.
                tiles = pair_exps.setdefault(p, {})
                todo = order if order is not None else \
                    [divmod(ti, 2) for ti in range(t0, t1)]
                for i in range(0, len(todo), 2):
                    for ch in todo[i:i + 2]:
                        _emit_scores_tile(p, ch, tiles)
                    yield

            def gen_ctx_group(p, hi, half):
                tiles = pair_exps[p]
                head = 2 * p + hi
                pc = ps_ctx.tile([NP, 512], F32, tag="ctx")
                for c in range(SC):
                    nc.tensor.matmul(
                        pc[:, :],
                        lhsT=vsb[:, c, head * NP:(head + 1) * NP],
                        rhs=tiles[(c, half)][:, hi * 512:(hi + 1) * 512],
                        start=(c == 0), stop=(c == SC - 1),
                    )
                    if c % 2 == 1:
                        yield
                # evacuate psum (frees the ctx ring slot), invert the
                # denominator row, broadcast it over 64 partitions with a
                # rank-1 bf16 matmul through the same psum ring, multiply.
                # reciprocal of the denominator row straight from psum row 0
                # (reciprocal_approx_fast requires an AP starting at
                # partition 0 -- satisfied by construction).
                rr = normp.tile([NP, 512], F32, tag="rr")
                with nc.allow_low_precision(reason="softmax denom recip"):
                    nc.vector.reciprocal_approx_fast(
                        out=rr[0:1, :], in_=pc[0:1, :])
                # broadcast the recip row over all partitions on the
                # (otherwise idle) GPSIMD engine -- keeps the rank-1 matmul
                # off TensorE and the extra tile off the ctx psum ring.
                bc = normp.tile([NP, 512], F32, tag="bc")
                nc.gpsimd.partition_broadcast(bc[:, :], rr[0:1, :], channels=NP)
                cu = normp.tile([NP, 512], F32, tag="cu")
                nc.vector.tensor_copy(out=cu[64:128, :], in_=pc[64:128, :])
                yield
                # ctx values sit at psum rows 64:128; head 2p+1 (hi=1) lands
                # there directly, head 2p is DMA-shifted down (DVE lanes are
                # partition-locked).
                n0 = half * 512
                if hi == 1:
                    dst = ctxT[64:128, p, n0:n0 + 512]
                    nc.vector.tensor_mul(out=dst, in0=cu[64:128, :],
                                         in1=bc[64:128, :])
                    if with_bias:
                        nc.vector.tensor_scalar_add(
                            out=dst, in0=dst,
                            scalar1=bvcol[64:128, head:head + 1])
                else:
                    tmp = normp.tile([NP, 512], BF16, tag="sh")
                    nc.vector.tensor_mul(
                        out=tmp[64:128, :], in0=cu[64:128, :],
                        in1=bc[64:128, :])
                    if with_bias:
                        nc.vector.tensor_scalar_add(
                            out=tmp[64:128, :], in0=tmp[64:128, :],
                            scalar1=bvcol[64:128, head:head + 1])
                    nc.sync.dma_start(
                        out=ctxT[0:64, p, n0:n0 + 512], in_=tmp[64:128, :])
                yield

            def gen_ctx(p, halves=(0, 1)):
                # interleave the two heads' groups so consecutive ctx matmuls
                # alternate psum banks (hides the same-bank accumulate
                # interlock).
                for half in halves:
                    g0 = gen_ctx_group(p, 0, half)
                    g1 = gen_ctx_group(p, 1, half)
                    for _ in zip(g0, g1):
                        yield

            def gen_outproj(ms):
                for m in ms:
                    po = ps_sc.tile([NP, S], F32, tag="sc")
                    for j in range(EC):
                        st, sp = (j == 0), (j == EC - 1)
                        nc.tensor.matmul(
                            po[:, 0:512],
                            lhsT=ctxT[:, j, m * NP:(m + 1) * NP],
                            rhs=wo_k[j][:, 0:512],
                            start=st, stop=sp,
                        )
                        nc.tensor.matmul(
                            po[:, 512:768],
                            lhsT=ctxT[:, j, m * NP:(m + 1) * NP],
                            rhs=wo_k[j][:, 512:768],
                            start=st, stop=sp,
                        )
                        yield
                    if with_bias:
                        o32 = outp.tile([NP, E], F32, tag="o32")
                        nc.vector.tensor_scalar_mul(o32, po[:, 0:768],
                                                    qm_sb[:, m:m + 1])
                        nc.vector.tensor_add(o32, o32, bo_bc)
                        osb = outp.tile([NP, E], F16, tag="osb")
                        nc.vector.tensor_copy(out=osb, in_=o32)
                    else:
                        osb = outp.tile([NP, E], F16, tag="osb")
                        nc.vector.tensor_scalar_mul(osb, po[:, 0:768],
                                                    qm_sb[:, m:m + 1])
                    nc.sync.dma_start(out=out[m * NP:(m + 1) * NP, :], in_=osb)
                    yield

            # ---- pipelined emission --------------------------------------
            # stage 0: warm up PE through the DMA wait; qkT(0) and qkT(6) on
            # separate rings so they overlap; then scores(0) with v(0) and
            # the stage-1 qkT prefetches. Each stage pre-emits the first two
            # scores tiles of the next pair so ScalarE never idles across
            # stage boundaries.
            _drive(gen_warmup())
            _drive(gen_qkT(0, ps_qk), gen_qkT(NPAIR, ps_sc))
            _drive_rot(gen_scores(0),
                       [_seq(gen_qkT(1, ps_qk), gen_qkT(NPAIR + 1, ps_qk),
                             gen_v2(0), gen_scores(1, 0, 4))],
                       per_round=3)
            # steady state: scores(p) || ctx(p-1) || qkT(p+1) prefetch || v.
            # The next pair's scores prefetch is chained AFTER its qkT
            # generators in one _seq: Tile resolves dependencies at emission
            # time, so the prefetch must be emitted after the qkT copies.
            for p in range(1, NPAIR):
                qk_work = [gen_qkT(p + 1, ps_qk), gen_qkT(NPAIR + p + 1, ps_qk)] \
                    if p + 1 < NPAIR else []
                if p <= 2:
                    qk_work.append(gen_v2(p))
                if p + 2 < NPAIR:
                    qk_work.append(gen_scores(p + 1, 0, 4))
                elif p + 2 == NPAIR:
                    # prefetch the last pair h-major so its half-0 ctx can
                    # run inside stage 5 and the tail shrinks
                    qk_work.append(gen_scores(
                        p + 1, order=[(0, 0), (1, 0), (2, 0), (3, 0)]))
                fillers = [gen_ctx(p - 1), _seq(*qk_work)]
                if p + 1 < NPAIR:
                    _drive_rot(gen_scores(p, 4), fillers, per_round=4)
                else:
                    # stage 5, phase A: remaining half-0 score tiles while
                    # ctx(4) drains
                    _drive_rot(
                        gen_scores(p, order=[(c, 0) for c in range(4, SC)]),
                        fillers, per_round=4)
                    # phase B: half-1 tiles paced against ctx(5, half 0)
                    # only (out-proj here would starve ScalarE)
                    _drive_rot(
                        gen_scores(p, order=[(c, 1) for c in range(SC)]),
                        [gen_ctx(NPAIR - 1, halves=(0,))], per_round=2)
            # tail: half-1 ctx overlapped with the m0-3 out-proj chains
            # (their ctxT inputs for pair 5 half 0 landed during phase B)
            _drive(gen_ctx(NPAIR - 1, halves=(1,)), gen_outproj(range(4)))
            _drive(gen_outproj(range(4, SC)))


def build_nc(with_bias=True):
    nc = bacc.Bacc()
    xp = nc.dram_tensor("xp", [NP, EC * S], BF16, kind="ExternalInput")
    wqk0 = nc.dram_tensor("wqk0", [NP, EC * 2 * NP], BF16, kind="ExternalInput")
    wqkr = nc.dram_tensor("wqkr", [NP, 5 * EC * 2 * NP], BF16, kind="ExternalInput")
    wvp = nc.dram_tensor("wvp", [NP, 3 * EC * 256], BF16, kind="ExternalInput")
    wop = nc.dram_tensor("wop", [NP, EC * E], BF16, kind="ExternalInput")
    bqk = nc.dram_tensor("bqkv", [3 * E], F32, kind="ExternalInput")
    bo = nc.dram_tensor("bo", [E], F32, kind="ExternalInput")
    kq = nc.dram_tensor("kqmask", [NP, 2 * SC], F32, kind="ExternalInput")
    out = nc.dram_tensor("out", [S, E], F16, kind="ExternalOutput")
    with tile.TileContext(nc) as tc:
        _body(tc, xp, wqk0, wqkr, wvp, wop, bqk, bo, kq, out, with_bias)
    nc.compile()
    return nc


def prep_in_maps(x, key_padding_mask, Wqkv_w, Wqkv_b, out_w, out_b):
    bf16 = ml_dtypes.bfloat16
    x = np.asarray(x, np.float32)
    mask = np.asarray(key_padding_mask).astype(bool)
    scale = 1.0 / np.sqrt(np.float32(D))

    wqkvT = np.asarray(Wqkv_w, np.float32).T.copy()      # (E, 3E), e-major
    wqkvT[:, :E] *= scale                                # fold 1/sqrt(d) into Wq
    bqkv = np.asarray(Wqkv_b, np.float32).copy()
    bqkv[:E] *= scale
    wotT = np.asarray(out_w, np.float32).T.copy()        # (E, E), e-major

    wqkvT = np.ascontiguousarray(wqkvT).astype(bf16)
    wotT = np.ascontiguousarray(wotT).astype(bf16)
    bo_ = np.asarray(out_b, np.float32)

    # pack weights into per-partition-contiguous layouts matching SBUF tiles:
    # wqk0 [128, (k 6, qk 2, c 128)] for pair 0; wqkr same per pair 1..5
    wq = wqkvT[:, 0:E].reshape(EC, NP, NPAIR, NP)        # (k, p, pair, c)
    wk = wqkvT[:, E:2 * E].reshape(EC, NP, NPAIR, NP)
    wqk = np.stack([wq, wk], axis=3)                     # (k, p, pair, qk, c)
    wqk = wqk.transpose(2, 1, 0, 3, 4)                   # (pair, p, k, qk, c)
    wqk0p = np.ascontiguousarray(wqk[0]).reshape(NP, -1)
    wqkrp = np.ascontiguousarray(wqk[1:].transpose(1, 0, 2, 3, 4)).reshape(NP, -1)
    # wvp [128, (g 3, k 6, c 256)]
    wv = wqkvT[:, 2 * E:].reshape(EC, NP, 3, 256)        # (k, p, g, c)
    wvp_ = np.ascontiguousarray(wv.transpose(1, 2, 0, 3)).reshape(NP, -1)
    # wop [128, (k 6, c 768)]
    wo = wotT.reshape(EC, NP, E)                         # (k, p, c)
    wopp = np.ascontiguousarray(wo.transpose(1, 0, 2)).reshape(NP, -1)

    in_maps = []
    for i in range(B):
        xti = np.ascontiguousarray(x[i].T).astype(bf16)  # (E, S)
        # xp [128, (k 6, s 1024)]
        xpp = np.ascontiguousarray(
            xti.reshape(EC, NP, S).transpose(1, 0, 2)).reshape(NP, -1)
        kbias = np.where(mask[i], 0.0, MASK_NEG).astype(np.float32)
        qmask = mask[i].astype(np.float32)
        kqm = np.concatenate(
            [kbias.reshape(8, 128).T, qmask.reshape(8, 128).T], axis=1
        ).astype(np.float32)
        in_maps.append(
            {
                "xp": xpp,
                "wqk0": wqk0p,
                "wqkr": wqkrp,
                "wvp": wvp_,
                "wop": wopp,
                "bqkv": bqkv,
                "bo": bo_,
                "kqmask": np.ascontiguousarray(kqm),
            }
        )
    return in_maps


_NC_CACHE = {}


def _get_nc(with_bias=True):
    if with_bias not in _NC_CACHE:
        _NC_CACHE[with_bias] = build_nc(with_bias)
    return _NC_CACHE[with_bias]


def kernel(x, key_padding_mask, Wqkv_w, Wqkv_b, out_w, out_b):
    in_maps = prep_in_maps(x, key_padding_mask, Wqkv_w, Wqkv_b, out_w, out_b)
    with_bias = bool(np.any(np.asarray(Wqkv_b) != 0) or np.any(np.asarray(out_b) != 0))
    nc = _get_nc(with_bias)
    res = run_bass_kernel_spmd(nc, in_maps, core_ids=list(range(B)))
    out = np.stack([res.results[i]["out"] for i in range(B)], axis=0)
    return out.astype(np.float32)


if __name__ == "__main__":
    nc = build_nc()
    print("build ok")
